# revision 41
# baseline (speedup 1.0000x reference)
"""Trainium2 Bass kernel: causal GQA self-attention with RoPE + QK RMS-norm.

Model (full): x[B=2,T=2048,C=2048] -> q/k/v proj -> RoPE -> RMSNorm(q,k) ->
causal GQA attention (16 q heads, 4 kv heads) -> out proj wproj.

Sharding over 8 NeuronCores: core = 4*b + g, b in {0,1} batch, g in {0..3}
kv-group. Each core handles one batch's kv head g and its 4 q heads
(h = 4g..4g+3), producing the partial c_proj output
y_heads @ wproj[:, 512g:512(g+1)].T of shape [T, C]. The host sums the 4
partials per batch (the "all-reduce after c_proj" done at gather time).

Device layouts (prepped on host):
  xT   [C, T]      x[b] transposed        (contraction dim c on partitions)
  wqT  [C, 512]    wq rows for 4 q heads, transposed
  wkvT [C, 256]    [wk_g ; wv_g] transposed
  wpT  [512, C]    wproj columns for the group, transposed
  cos/sin [T, 128] rope tables
  mask [128,128]   upper-tri (tk<=tq) 0/1 for diagonal blocks
  ident [128,128]  identity for PE transposes

In-kernel dataflow per core (all matmuls fp32r, PSUM f32 accumulate):
  stage A per t-tile: Q/K/V projections (lhsT = xT tile, moving = weights),
    RoPE + RMSNorm on natural [t,d] tiles, PE-transpose normalized Q/K to
    [d,t] layout.
  stage B per (tq-slice j, head h): S^T = K_tile^T . Q  -> exp on ACT ->
    mask diag -> denominators via ones-matmul -> AV accumulate (Y^T).
  stage C per j: c_proj with lhsT = normalized Y^T, natural [t, e] output.
Causality: tk-tile i contributes to tq-slice j only for i <= 4j+3; partial
blocks restrict to the valid column suffix.
"""

import math
from contextlib import ExitStack

import numpy as np

import concourse.bass as bass
import concourse.mybir as mybir
import concourse.tile as tile
from concourse import bacc
from concourse.bass import ts
from concourse.bass_utils import run_bass_kernel_spmd

F32 = mybir.dt.float32
N_HEAD = 16
N_KV = 4
D = 128
RMS_EPS = float(np.finfo(np.float32).eps)
SCALE = 1.0 / math.sqrt(D)


def build_bass(T=2048, C=2048, HQ=4, E=2048, rep=1, dt=mybir.dt.float32r,
               stages='ABC', dt_proj=None, dt_att=None, dt_cproj=None,
               pscfg=(3, 2, 2, 1, "s"), pbufs=3, obufs=2, bbufs=2):
    """One core's program. T,C,E multiples of 512; HQ q-heads (1 kv head)."""
    TT, CT, NE, TQ = T // 128, C // 128, E // 512, T // 512
    HD = HQ * 128
    dt_proj = dt_proj or dt
    dt_att = dt_att or dt
    dt_cproj = dt_cproj or dt

    nc = bacc.Bacc("TRN2", target_bir_lowering=False)
    xT_d = nc.dram_tensor("xT", [C, T], dt_proj, kind="ExternalInput")
    wqT_d = nc.dram_tensor("wqT", [C, HD], dt_proj, kind="ExternalInput")
    wkvT_d = nc.dram_tensor("wkvT", [C, 256], dt_proj, kind="ExternalInput")
    wpT_d = nc.dram_tensor("wpT", [HD, E], dt_cproj, kind="ExternalInput")
    cos_d = nc.dram_tensor("cosd", [T, D], F32, kind="ExternalInput")
    sin_d = nc.dram_tensor("sind", [T, D], F32, kind="ExternalInput")
    mask_d = nc.dram_tensor("maskd", [128, 128], dt_att, kind="ExternalInput")
    id_d = nc.dram_tensor("identd", [128, 128], dt_att, kind="ExternalInput")
    out_d = nc.dram_tensor("out", [T, E], F32, kind="ExternalOutput")

    with tile.TileContext(nc) as tc, ExitStack() as ctx:
        P = lambda **kw: ctx.enter_context(tc.tile_pool(**kw))
        wp = P(name="w", bufs=1)            # persistent weights/constants
        xp = P(name="x", bufs=2)            # xT strips
        csp = P(name="cs", bufs=2)          # cos/sin tiles
        rp = P(name="rope", bufs=2)         # rope scratch
        qnp = P(name="qn", bufs=2)          # normalized q/k (pre-transpose)
        pp = P(name="p", bufs=pbufs)        # exp(P) tiles
        bp = P(name="bc", bufs=bbufs)       # denominators / bcast
        yp = P(name="y", bufs=1)            # per-j YT
        op = P(name="o", bufs=obufs)        # output staging
        sb_, tb_, ab_, db_, proj_tag = pscfg
        ps_s = P(name="ps_s", bufs=sb_, space="PSUM")   # proj / scores / cproj
        ps_t = P(name="ps_t", bufs=tb_, space="PSUM")   # transposes
        ps_a = P(name="ps_a", bufs=ab_, space="PSUM")   # AV accumulators
        ps_d = P(name="ps_d", bufs=db_, space="PSUM")   # denominators
        ps = {"ps_s": ps_s, "ps_t": ps_t, "ps_a": ps_a, "ps_d": ps_d}

        # persistent SBUF
        wq_s = wp.tile([128, CT, HD], dt_proj)
        nc.sync.dma_start(wq_s, wqT_d.ap().rearrange("(n p) m -> p n m", p=128))
        wkv_s = wp.tile([128, CT, 256], dt_proj)
        nc.sync.dma_start(wkv_s, wkvT_d.ap().rearrange("(n p) m -> p n m", p=128))
        wp_s = wp.tile([128, HQ, E], dt_cproj)
        nc.sync.dma_start(wp_s, wpT_d.ap().rearrange("(n p) m -> p n m", p=128))
        mask_s = wp.tile([128, 128], dt_att)
        nc.sync.dma_start(mask_s, mask_d.ap())
        ident = wp.tile([128, 128], dt_att)
        nc.sync.dma_start(ident, id_d.ap())
        def ones_tile(shape, name):
            t = wp.tile(shape, dt_att, name=name, tag=name)
            if dt_att == mybir.dt.float32r:
                nc.vector.memset(t.bitcast(mybir.dt.uint32), 0x3F800000)
            else:
                nc.vector.memset(t, 1.0)
            return t
        ones_c = ones_tile([128, 1], "ones_c")
        ones_r = ones_tile([1, 128], "ones_r")
        eps_s = wp.tile([128, 1], F32)
        nc.vector.memset(eps_s, RMS_EPS)

        def bcast(ap, axis, n):
            """Insert a stride-0 dim of size n at free-axis position `axis`."""
            a = list(ap.ap)
            a.insert(axis, [0, n])
            return bass.AP(tensor=ap.tensor, offset=ap.offset, ap=a)

        for _ in range(rep):
            # persistent-per-rep activation tiles (distinct tags)
            qT = {}  # (h, j) -> [128, 4, 128] tile, d-major
            kT = []  # i -> [128, 128]
            vS = []  # i -> [128, 128]
            for h in range(HQ):
                for j in range(TQ):
                    qT[(h, j)] = wp.tile([128, 4, 128], dt_att, tag=f"qT{h}_{j}", name=f"qT{h}_{j}")
            for i in range(TT):
                kT.append(wp.tile([128, 128], dt_att, tag=f"kT{i}", name=f"kT{i}"))
                vS.append(wp.tile([128, 128], dt_att, tag=f"vS{i}", name=f"vS{i}"))

            # ---- stage A: projections + rope + rms + transpose ----
            xT_r = xT_d.ap().rearrange("(n p) t -> p n t", p=128)
            for i in range(TT):
                xs = xp.tile([128, CT, 128], dt_proj)
                nc.sync.dma_start(xs, xT_r[:, :, ts(i, 128)])
                cst = csp.tile([128, D], F32, tag="cos")
                nc.sync.dma_start(cst, cos_d.ap()[ts(i, 128), :])
                snt = csp.tile([128, D], F32, tag="sin")
                nc.sync.dma_start(snt, sin_d.ap()[ts(i, 128), :])

                pq = ps['ps_s'].tile([128, HD], F32, tag=proj_tag, bufs=(None if proj_tag == "s" else 2))
                for c in range(CT):
                    nc.tensor.matmul(pq, xs[:, c], wq_s[:, c],
                                     start=(c == 0), stop=(c == CT - 1))
                pkv = ps['ps_s'].tile([128, 256], F32, tag=proj_tag, bufs=(None if proj_tag == "s" else 2))
                for c in range(CT):
                    nc.tensor.matmul(pkv, xs[:, c], wkv_s[:, c],
                                     start=(c == 0), stop=(c == CT - 1))
                nc.scalar.copy(vS[i], pkv[:, 128:256])

                def rope_rms(src, nh, dst_list):
                    """src: psum AP viewed [128, nh, 128]; writes dt tiles."""
                    ro = rp.tile([128, nh, D], F32, tag=f"ro{nh}")
                    nc.vector.tensor_mul(ro, src, bcast(cst[:, :], 1, nh))
                    tmp = rp.tile([128, nh, 64], F32, tag=f"tm{nh}")
                    nc.vector.tensor_mul(tmp, src[:, :, 64:128],
                                         bcast(snt[:, 0:64], 1, nh))
                    nc.vector.tensor_sub(ro[:, :, 0:64], ro[:, :, 0:64], tmp)
                    tmp2 = rp.tile([128, nh, 64], F32, tag=f"t2{nh}")
                    nc.vector.tensor_mul(tmp2, src[:, :, 0:64],
                                         bcast(snt[:, 64:128], 1, nh))
                    nc.vector.tensor_add(ro[:, :, 64:128], ro[:, :, 64:128], tmp2)
                    sq = rp.tile([128, nh], F32, tag=f"sq{nh}")
                    scr = rp.tile([128, nh, D], F32, tag=f"sc{nh}")
                    nc.vector.tensor_mul(scr, ro, ro)
                    nc.vector.reduce_sum(sq, scr, axis=mybir.AxisListType.X)
                    rs = rp.tile([128, nh], F32, tag=f"rs{nh}")
                    nc.scalar.activation(rs, sq,
                                         mybir.ActivationFunctionType.Sqrt,
                                         bias=eps_s[:, :], scale=1.0 / D)
                    rr = rp.tile([128, nh], F32, tag=f"rr{nh}")
                    nc.vector.reciprocal(rr, rs)
                    qn = qnp.tile([128, nh, D], dt_att, tag=f"qn{nh}")
                    for h in range(nh):
                        nc.vector.tensor_scalar_mul(qn[:, h], ro[:, h],
                                                    rr[:, h:h + 1])
                    for h in range(nh):
                        pt = ps_t.tile([128, 128], dt_att)
                        nc.tensor.transpose(pt, qn[:, h], ident)
                        nc.scalar.copy(dst_list[h], pt)

                j, tsub = i // 4, i % 4
                rope_rms(pq[:].rearrange("p (h d) -> p h d", d=D), HQ,
                         [qT[(h, j)][:, tsub] for h in range(HQ)])
                rope_rms(pkv[:, 0:128].rearrange("p (h d) -> p h d", d=D), 1,
                         [kT[i]])

            # ---- stage B + C per tq-slice ----
            if stages == 'A':
                dbg = op.tile([128, 512], F32, tag="ot")
                nc.vector.tensor_copy(dbg[:, 0:128], kT[0])
                nc.sync.dma_start(out_d.ap()[0:128, 0:512], dbg)
                continue
            for j in range(TQ):
                ynj = yp.tile([128, HQ, 4, 128], dt_cproj)
                for h in range(HQ):
                    nblk = 4 * j + 4
                    pav = ps['ps_a'].tile([128, 512], F32)
                    pd = ps['ps_d'].tile([1, 512], F32)
                    for i in range(nblk):
                        ai = max(0, i - 4 * j) * 128
                        psb = ps['ps_s'].tile([128, 512], F32, tag="s")
                        nc.tensor.matmul(psb[:, ai:512], kT[i],
                                         qT[(h, j)][:, ai // 128:4])
                        pe = pp.tile([128, 512], dt_att)
                        nc.scalar.activation(pe[:, ai:512], psb[:, ai:512],
                                             mybir.ActivationFunctionType.Exp,
                                             scale=SCALE)
                        if i >= 4 * j:
                            nc.vector.tensor_mul(pe[:, ai:ai + 128],
                                                 pe[:, ai:ai + 128], mask_s)
                        nc.tensor.matmul(pd[:, ai:512], ones_c, pe[:, ai:512],
                                         start=(i == 0), stop=(i == nblk - 1))
                        nc.tensor.matmul(pav[:, ai:512], vS[i], pe[:, ai:512],
                                         start=(i == 0), stop=(i == nblk - 1))
                    rd = bp.tile([1, 512], F32, tag="rd")
                    nc.vector.reciprocal(rd, pd)
                    rdr = bp.tile([1, 512], dt_att, tag="rdr")
                    nc.scalar.copy(rdr, rd)
                    pb = ps['ps_s'].tile([128, 512], F32, tag="s")
                    nc.tensor.matmul(pb, ones_r, rdr)
                    bc = bp.tile([128, 512], F32, tag="bc")
                    nc.scalar.copy(bc, pb)
                    nc.vector.tensor_mul(
                        ynj[:, h].rearrange("p a b -> p (a b)"), pav, bc)
                if stages == 'AB':
                    dbg2 = op.tile([128, 512], F32, tag="ot")
                    nc.vector.tensor_copy(dbg2, ynj[:, 0].rearrange("p a b -> p (a b)"))
                    nc.sync.dma_start(out_d.ap()[ts(j, 128), 0:512], dbg2)
                    continue
                for tsub in range(4):
                    for e in range(NE):
                        pc = ps['ps_s'].tile([128, 512], F32, tag="s")
                        for h in range(HQ):
                            nc.tensor.matmul(pc, ynj[:, h, tsub],
                                             wp_s[:, h, ts(e, 512)],
                                             start=(h == 0), stop=(h == HQ - 1))
                        ot = op.tile([128, 512], F32, tag="ot")
                        nc.scalar.copy(ot, pc)
                        nc.sync.dma_start(
                            out_d.ap()[512 * j + 128 * tsub:
                                       512 * j + 128 * tsub + 128,
                                       ts(e, 512)], ot)
    nc.compile()
    return nc


BF16 = mybir.dt.bfloat16


def build_bass2(T=2048, C=2048, HQ=4, E=2048, rep=1,
                pscfg2=(3, 2, 3, 2, 1, 2), pbufs=6, obufs=6, ybufs=1,
                csbufs=2, qsbufs=2, ot_split=2, interleave=False,
                dma_split=False, rms_batch=False, rpbufs=2):
    """v2: bf16 end-to-end, device-side transposes, interleaved stages.

    Per-core program for one (batch, kv-group): QKV proj -> RoPE+RMS ->
    causal GQA attention (4 q heads, 1 kv head) -> partial c_proj.
    Layouts: x arrives natural [T, C] bf16 and is transposed on-device
    via the DMA xbar; q/k are transposed to [d, t] the same way. cos and
    sign-folded sin (s2) arrive bf16 so RoPE runs in DVE 2x mode.
    """
    TT, CT, NE, TQ = T // 128, C // 128, E // 512, T // 512
    HD = HQ * 128

    nc = bacc.Bacc("TRN2", target_bir_lowering=False)
    x_d = nc.dram_tensor("xt3", [128, TQ * CT * 512], BF16,
                         kind="ExternalInput")
    # weights / tables arrive pre-tiled [128, ...] so each load is one
    # 128-descriptor contiguous DMA (HWDGE descriptor gen is the SP-seq
    # bottleneck otherwise)
    wqT_d = nc.dram_tensor("wqT", [128, CT * HD], BF16, kind="ExternalInput")
    wkvT_d = nc.dram_tensor("wkvT", [128, CT * 256], BF16,
                            kind="ExternalInput")
    wpT_d = nc.dram_tensor("wpT", [128, HQ * E], BF16, kind="ExternalInput")
    cos_d = nc.dram_tensor("cosb", [128, TT * D], BF16, kind="ExternalInput")
    s2_d = nc.dram_tensor("s2b", [128, TT * D], BF16, kind="ExternalInput")
    mask_d = nc.dram_tensor("maskd", [128, 128], BF16, kind="ExternalInput")
    out_d = nc.dram_tensor("out", [T, E], BF16, kind="ExternalOutput")

    with tile.TileContext(nc) as tc, ExitStack() as ctx:
        P = lambda **kw: ctx.enter_context(tc.tile_pool(**kw))
        wp = P(name="w", bufs=1)
        qsp = P(name="qs", bufs=qsbufs)     # PSUM->SBUF bf16 proj drains
        rp = P(name="rope", bufs=rpbufs)
        qnp = P(name="qn", bufs=2)
        pp = P(name="p", bufs=pbufs)        # exp(P) tiles
        bp = P(name="bc", bufs=2)
        yp = P(name="y", bufs=ybufs)
        op = P(name="o", bufs=obufs)
        if len(pscfg2) == 5:
            pscfg2 = tuple(pscfg2) + (2,)
        b_pq, b_pkv, b_s, b_av, b_d, b_c = pscfg2
        ps = {}  # phase-scoped PSUM pools (see the rep loop)

        # persistent weights; x^T arrives host-pre-tiled per tq-slice so
        # each slice is one contiguous 128-descriptor DMA. wq is split in
        # c-halves so the first projection starts after ~1 MB of DMA.
        wq_h = [wp.tile([128, CT // 2, HD], BF16, tag=f"wqh{h}",
                        name=f"wqh{h}") for h in range(2)]
        wkv_s = wp.tile([128, CT, 256], BF16)
        wp_s = wp.tile([128, HQ, E], BF16)
        cos_s = wp.tile([128, TT, D], BF16)
        s2_s = wp.tile([128, TT, D], BF16)
        mask_s = wp.tile([128, 128], BF16)
        ones_c = wp.tile([128, 1], BF16, name="ones_c", tag="ones_c")
        nc.vector.memset(ones_c, 1.0)
        ones_rb = wp.tile([1, 128], BF16, name="ones_rb", tag="ones_rb")
        nc.vector.memset(ones_rb, 1.0)
        eps_s = wp.tile([128, 1], F32)
        nc.vector.memset(eps_s, RMS_EPS)
        xs_j = [[wp.tile([128, CT // 2, 512], BF16, tag=f"xsj{jj}_{h}",
                         name=f"xsj{jj}_{h}") for h in range(2)]
                for jj in range(TQ)]

        def bcast(ap, axis, n):
            a = list(ap.ap)
            a.insert(axis, [0, n])
            return bass.AP(tensor=ap.tensor, offset=ap.offset, ap=a)

        NH = HQ + 1  # q heads + the kv head, roped/normed together

        def stage_x(j):
            eng = nc.scalar if dma_split else nc.sync
            for h in range(2):
                eng.dma_start(
                    xs_j[j][h].rearrange("p a b -> p (a b)"),
                    x_d.ap()[:, j * CT * 512 + h * (CT // 2) * 512:
                             j * CT * 512 + (h + 1) * (CT // 2) * 512])

        def stage_A(i, qT, kT, vS, ro4, sq4):
            tsub = i % 4
            HC = CT // 2
            pq = ps['ps_pq'].tile([128, HD], F32, tag="pq")
            for c in range(CT):
                nc.tensor.matmul(pq,
                                 xs_j[i // 4][c // HC][:, c % HC,
                                                       ts(i % 4, 128)],
                                 wq_h[c // HC][:, c % HC], start=(c == 0),
                                 stop=(c == CT - 1))
            pkv = ps['ps_pk'].tile([128, 256], F32, tag="pkv")
            for c in range(CT):
                nc.tensor.matmul(pkv,
                                 xs_j[i // 4][c // HC][:, c % HC,
                                                       ts(i % 4, 128)],
                                 wkv_s[:, c], start=(c == 0),
                                 stop=(c == CT - 1))
            # drain PSUM quickly: bf16 copies, q heads + k into one tile
            qk = qsp.tile([128, NH, D], BF16, tag="qk")
            nc.scalar.copy(
                qk[:, 0:HQ].rearrange("p h d -> p (h d)"), pq)
            nc.scalar.copy(qk[:, HQ], pkv[:, 0:D])
            nc.scalar.copy(vS[i], pkv[:, D:2 * D])

            # RoPE (s2 carries the rotate-half signs); squared-norm sums
            # land in sq4, the sqrt happens once per 4-tile group.
            ro = ro4[tsub]
            nc.vector.tensor_mul(ro, qk, bcast(cos_s[:, i], 1, NH))
            tml = rp.tile([128, NH, 64], BF16, tag="tml")
            nc.vector.tensor_mul(tml, qk[:, :, 64:128],
                                 bcast(s2_s[:, i, 0:64], 1, NH))
            nc.vector.tensor_add(ro[:, :, 0:64], ro[:, :, 0:64], tml)
            tmh = rp.tile([128, NH, 64], BF16, tag="tmh")
            nc.vector.tensor_mul(tmh, qk[:, :, 0:64],
                                 bcast(s2_s[:, i, 64:128], 1, NH))
            nc.vector.tensor_add(ro[:, :, 64:128], ro[:, :, 64:128], tmh)
            scr = rp.tile([128, NH, D], BF16, tag="scr")
            nc.vector.tensor_mul(scr, ro, ro)
            nc.vector.reduce_sum(sq4[:, tsub], scr, axis=mybir.AxisListType.X)
            if not rms_batch:
                j = i // 4
                rs = rp.tile([128, NH], F32, tag="rs")
                nc.scalar.activation(rs, sq4[:, tsub],
                                     mybir.ActivationFunctionType.Sqrt,
                                     bias=eps_s[:, :], scale=1.0 / D)
                rr = rp.tile([128, NH], F32, tag="rr")
                nc.vector.reciprocal(rr, rs)
                qn = qnp.tile([128, NH, D], BF16, tag="qn")
                for h in range(NH):
                    nc.vector.tensor_scalar_mul(qn[:, h], ro[:, h],
                                                rr[:, h:h + 1])
                for h in range(HQ):
                    nc.sync.dma_start_transpose(qT[(h, j)][:, tsub], qn[:, h])
                nc.sync.dma_start_transpose(kT[i], qn[:, NH - 1])

        def stage_A_norm(j, qT, kT, ro4, sq4):
            """One sqrt for the whole 4-tile group, then scale + transpose."""
            rs4 = rp.tile([128, 4, NH], F32, tag="rs4")
            nc.scalar.activation(rs4.rearrange("p a b -> p (a b)"),
                                 sq4.rearrange("p a b -> p (a b)"),
                                 mybir.ActivationFunctionType.Sqrt,
                                 bias=eps_s[:, :], scale=1.0 / D)
            rr4 = rp.tile([128, 4, NH], F32, tag="rr4")
            nc.vector.reciprocal(rr4.rearrange("p a b -> p (a b)"),
                                 rs4.rearrange("p a b -> p (a b)"))
            for tsub in range(4):
                qn = qnp.tile([128, NH, D], BF16, tag="qn")
                for h in range(NH):
                    nc.vector.tensor_scalar_mul(qn[:, h], ro4[tsub][:, h],
                                                rr4[:, tsub, h:h + 1])
                for h in range(HQ):
                    nc.sync.dma_start_transpose(qT[(h, j)][:, tsub], qn[:, h])
                nc.sync.dma_start_transpose(kT[4 * j + tsub], qn[:, NH - 1])

        def stage_B(j, qT, kT, vS, ynj):
            pending = []

            def normalize(h, pav, pd):
                rd = bp.tile([1, 512], F32, tag="rd")
                nc.vector.reciprocal(rd, pd)
                rdr = bp.tile([1, 512], BF16, tag="rdr")
                nc.vector.tensor_copy(rdr, rd)
                pb = ps['ps_s'].tile([128, 512], F32, tag="s")
                nc.tensor.matmul(pb, ones_rb, rdr)
                bc = bp.tile([128, 512], F32, tag="bc")
                nc.vector.tensor_copy(bc, pb)
                nc.vector.tensor_mul(
                    ynj[:, h].rearrange("p a b -> p (a b)"), pav, bc)

            for h in range(HQ):
                nblk = 4 * j + 4
                pav = ps['ps_a'].tile([128, 512], F32)
                pd = ps['ps_d'].tile([1, 512], F32)
                for i in range(nblk):
                    ai = max(0, i - 4 * j) * 128
                    psb = ps['ps_s'].tile([128, 512], F32, tag="s")
                    nc.tensor.matmul(psb[:, ai:512], kT[i],
                                     qT[(h, j)][:, ai // 128:4])
                    if i == 0 and pending:
                        # emit the previous head's normalize chain here so
                        # its PE broadcast lands behind this head's ready
                        # score matmuls instead of stalling the stream
                        normalize(*pending.pop())
                    pe = pp.tile([128, 512], BF16)
                    nc.scalar.activation(pe[:, ai:512], psb[:, ai:512],
                                         mybir.ActivationFunctionType.Exp,
                                         scale=SCALE)
                    if i >= 4 * j:
                        nc.vector.tensor_mul(pe[:, ai:ai + 128],
                                             pe[:, ai:ai + 128], mask_s)
                    nc.tensor.matmul(pd[:, ai:512], ones_c, pe[:, ai:512],
                                     start=(i == 0), stop=(i == nblk - 1))
                    nc.tensor.matmul(pav[:, ai:512], vS[i], pe[:, ai:512],
                                     start=(i == 0), stop=(i == nblk - 1))
                pending.append((h, pav, pd))
            normalize(*pending.pop())

        def stage_C(j, ynj):
            for tsub in range(4):
                for e in range(NE):
                    if 'ps_c' in ps:
                        pc = ps['ps_c'].tile([128, 512], F32, tag="pc",
                                             name="pc")
                    else:
                        pc = ps['ps_s'].tile([128, 512], F32, tag="s",
                                             name="pc")
                    for h in range(HQ):
                        nc.tensor.matmul(pc, ynj[:, h, tsub],
                                         wp_s[:, h, ts(e, 512)],
                                         start=(h == 0), stop=(h == HQ - 1))
                    ot = op.tile([128, 512], BF16, tag="ot")
                    if (tsub * NE + e) % ot_split == 0:
                        nc.scalar.copy(ot, pc)
                    else:
                        nc.vector.tensor_copy(ot, pc)
                    eng = (nc.scalar if dma_split and (tsub * NE + e) % 2
                           else nc.sync)
                    eng.dma_start(
                        out_d.ap()[512 * j + 128 * tsub:
                                   512 * j + 128 * tsub + 128,
                                   ts(e, 512)], ot)

        rep_done = []
        for _ in range(rep):
            qT = {}
            kT, vS = [], []
            for h in range(HQ):
                for j in range(TQ):
                    qT[(h, j)] = wp.tile([128, 4, 128], BF16,
                                         tag=f"qT{h}_{j}", name=f"qT{h}_{j}")
            for i in range(TT):
                kT.append(wp.tile([128, 128], BF16, tag=f"kT{i}",
                                  name=f"kT{i}"))
                vS.append(wp.tile([128, 128], BF16, tag=f"vS{i}",
                                  name=f"vS{i}"))
            first_rep = not rep_done
            if first_rep:
                rep_done.append(1)
                # preamble in demand order: first wq/x0 halves so the
                # projections start early, then kv/rope tables, then the
                # rest of x, then the c_proj weight
                HW_ = CT // 2 * HD
                for h in range(2):
                    nc.sync.dma_start(
                        wq_h[h].rearrange("p a b -> p (a b)"),
                        wqT_d.ap()[:, h * HW_:(h + 1) * HW_])
                    nc.sync.dma_start(
                        xs_j[0][h].rearrange("p a b -> p (a b)"),
                        x_d.ap()[:, h * (CT // 2) * 512:
                                 (h + 1) * (CT // 2) * 512])
                nc.sync.dma_start(wkv_s.rearrange("p a b -> p (a b)"),
                                  wkvT_d.ap())
                nc.sync.dma_start(cos_s.rearrange("p a b -> p (a b)"),
                                  cos_d.ap())
                nc.sync.dma_start(s2_s.rearrange("p a b -> p (a b)"),
                                  s2_d.ap())
                nc.sync.dma_start(mask_s, mask_d.ap())
                for j in range(1, TQ):
                    stage_x(j)
                nc.sync.dma_start(wp_s.rearrange("p a b -> p (a b)"),
                                  wpT_d.ap())

            def a_group(j):
                if not first_rep or j > 0:
                    pass
                if not first_rep:
                    stage_x(j)
                ro4 = [rp.tile([128, NH, D], BF16, tag=f"ro{t}",
                               name=f"ro{t}")
                       for t in range(4)]
                sq4 = rp.tile([128, 4, NH], F32, tag="sq4")
                for i in range(4 * j, 4 * j + 4):
                    stage_A(i, qT, kT, vS, ro4, sq4)
                if rms_batch:
                    stage_A_norm(j, qT, kT, ro4, sq4)

            if interleave:
                ps["ps_pq"] = ctx.enter_context(
                    tc.tile_pool(name="ps_pq", bufs=b_pq, space="PSUM"))
                ps["ps_pk"] = ctx.enter_context(
                    tc.tile_pool(name="ps_pk", bufs=b_pkv, space="PSUM"))
                ps["ps_s"] = ctx.enter_context(
                    tc.tile_pool(name="ps_s", bufs=b_s, space="PSUM"))
                ps["ps_a"] = ctx.enter_context(
                    tc.tile_pool(name="ps_a", bufs=b_av, space="PSUM"))
                ps["ps_d"] = ctx.enter_context(
                    tc.tile_pool(name="ps_d", bufs=b_d, space="PSUM"))
                for j in range(TQ):
                    a_group(j)
                    ynj = yp.tile([128, HQ, 4, 128], BF16, tag="ynj")
                    stage_B(j, qT, kT, vS, ynj)
                    stage_C(j, ynj)
            else:
                # phase-scoped PSUM pools: projections get b_pq+b_pkv banks,
                # then attention/c_proj reuse them as b_s+b_av+b_d banks
                with tc.tile_pool(name="ps_pq", bufs=b_pq, space="PSUM") \
                        as ps["ps_pq"], \
                        tc.tile_pool(name="ps_pk", bufs=b_pkv, space="PSUM") \
                        as ps["ps_pk"]:
                    for j in range(TQ):
                        a_group(j)
                with tc.tile_pool(name="ps_s", bufs=b_s, space="PSUM") \
                        as ps["ps_s"], \
                        tc.tile_pool(name="ps_a", bufs=b_av, space="PSUM") \
                        as ps["ps_a"], \
                        tc.tile_pool(name="ps_d", bufs=b_d, space="PSUM") \
                        as ps["ps_d"], \
                        tc.tile_pool(name="ps_c", bufs=b_c, space="PSUM") \
                        as ps["ps_c"]:
                    for j in range(TQ):
                        ynj = yp.tile([128, HQ, 4, 128], BF16,
                                      tag=f"ynj{j}", name=f"ynj{j}")
                        stage_B(j, qT, kT, vS, ynj)
                        stage_C(j, ynj)
    nc.compile()
    return nc


def make_core_inputs2(x, cos, sin, wq, wk, wv, wproj):
    """Full inputs -> 8 per-core bf16 input dicts (no host transposes)."""
    import ml_dtypes
    bf = ml_dtypes.bfloat16
    x = np.asarray(x, np.float32)
    B = x.shape[0]
    T = x.shape[1]
    TT = T // 128

    def ptile(a, p=128):
        """[N*p, M] -> [p, N*M] so each partition's data is contiguous."""
        n = a.shape[0] // p
        return np.ascontiguousarray(
            a.reshape(n, p, a.shape[1]).transpose(1, 0, 2).reshape(p, -1))

    cos2 = np.asarray(cos, np.float32).reshape(-1, D)
    sin2 = np.asarray(sin, np.float32).reshape(-1, D)
    s2 = np.concatenate([-sin2[:, :64], sin2[:, 64:]], axis=1)
    cosb = ptile(cos2).astype(bf)
    s2b = ptile(s2).astype(bf)
    wq = np.asarray(wq, np.float32)
    wk = np.asarray(wk, np.float32)
    wv = np.asarray(wv, np.float32)
    wproj = np.asarray(wproj, np.float32)
    mask = np.triu(np.ones((128, 128), np.float32)).astype(bf)
    TQ, CT = T // 512, x.shape[2] // 128
    xb = []
    for b in range(B):
        # [T, C] -> [128, TQ, CT, 512]: elem (p, j, cc, t') = x[512j+t', 128cc+p]
        xt3 = x[b].reshape(TQ, 512, CT, 128).transpose(3, 0, 2, 1)
        xb.append(np.ascontiguousarray(xt3).reshape(128, -1).astype(bf))
    in_maps = []
    for b in range(B):
        for g in range(N_KV):
            wqT = ptile(np.ascontiguousarray(
                wq[512 * g:512 * g + 512].T)).astype(bf)
            wkvT = ptile(np.ascontiguousarray(np.concatenate(
                [wk[128 * g:128 * g + 128],
                 wv[128 * g:128 * g + 128]], axis=0).T)).astype(bf)
            wpT = ptile(np.ascontiguousarray(
                wproj[:, 512 * g:512 * g + 512].T)).astype(bf)
            in_maps.append({
                "xt3": xb[b], "wqT": wqT, "wkvT": wkvT, "wpT": wpT,
                "cosb": cosb, "s2b": s2b, "maskd": mask,
            })
    return in_maps


def make_core_inputs_v1(x, cos, sin, wq, wk, wv, wproj,
                        np_proj=np.float32, np_att=np.float32,
                        np_cproj=np.float32):
    """Full inputs -> list of 8 per-core input dicts (host-side sharding)."""
    x = np.asarray(x, dtype=np.float32)
    cos2 = np.ascontiguousarray(np.asarray(cos, np.float32).reshape(-1, D))
    sin2 = np.ascontiguousarray(np.asarray(sin, np.float32).reshape(-1, D))
    wq = np.asarray(wq, np.float32)
    wk = np.asarray(wk, np.float32)
    wv = np.asarray(wv, np.float32)
    wproj = np.asarray(wproj, np.float32)
    B = x.shape[0]
    mask = np.triu(np.ones((128, 128), np.float32)).astype(np_att)
    ident = np.eye(128, dtype=np.float32).astype(np_att)
    in_maps = []
    xTs = [np.ascontiguousarray(x[b].T).astype(np_proj) for b in range(B)]
    for b in range(B):
        for g in range(N_KV):
            wqT = np.ascontiguousarray(wq[512 * g:512 * g + 512].T).astype(np_proj)
            wkvT = np.ascontiguousarray(
                np.concatenate([wk[128 * g:128 * g + 128],
                                wv[128 * g:128 * g + 128]], axis=0).T).astype(np_proj)
            wpT = np.ascontiguousarray(
                wproj[:, 512 * g:512 * g + 512].T).astype(np_cproj)
            in_maps.append({
                "xT": xTs[b], "wqT": wqT, "wkvT": wkvT, "wpT": wpT,
                "cosd": cos2, "sind": sin2, "maskd": mask, "identd": ident,
            })
    return in_maps


make_core_inputs = make_core_inputs2

_NC_CACHE = {}
_RUNNER_CACHE = {}


class _Runner:
    """Compile the Bass module once into a sharded PJRT executable.

    Per-call cost after the first call: H2D of the sharded inputs, an
    on-device zero-fill for the donated output buffers, one execute,
    D2H of the outputs. No re-trace / re-compile per call (the stock
    run_bass_kernel_spmd path re-jits every call).
    """

    def __init__(self, nc, n_cores=8):
        import jax
        from jax.sharding import Mesh, PartitionSpec, NamedSharding
        from jax.experimental.shard_map import shard_map
        from concourse import bass2jax

        bass2jax.install_neuronx_cc_hook()
        self.jax = jax
        self.nc = nc
        self.n_cores = n_cores
        partition_name = (
            nc.partition_id_tensor.name if nc.partition_id_tensor else None
        )
        in_names, out_names, out_avals = [], [], []
        for alloc in nc.m.functions[0].allocations:
            if not isinstance(alloc, mybir.MemoryLocationSet):
                continue
            name = alloc.memorylocations[0].name
            if alloc.kind == "ExternalInput":
                if name != partition_name:
                    in_names.append(name)
            elif alloc.kind == "ExternalOutput":
                out_names.append(name)
                shape = tuple(alloc.tensor_shape)
                out_avals.append(
                    jax.core.ShapedArray(shape, mybir.dt.np(alloc.dtype)))
        self.in_names, self.out_names, self.out_avals = \
            in_names, out_names, out_avals
        n_params, n_outs = len(in_names), len(out_names)
        all_in_names = list(in_names) + list(out_names)
        if partition_name is not None:
            all_in_names.append(partition_name)

        def _body(*args):
            operands = list(args)
            if partition_name is not None:
                operands.append(bass2jax.partition_id_tensor())
            return tuple(bass2jax._bass_exec_p.bind(
                *operands,
                out_avals=tuple(out_avals),
                in_names=tuple(all_in_names),
                out_names=tuple(out_names),
                lowering_input_output_aliases=(),
                sim_require_finite=True,
                sim_require_nnan=True,
                nc=nc,
            ))

        devices = jax.devices()[:n_cores]
        self.mesh = Mesh(np.array(devices), ("core",))
        self.sharding = NamedSharding(self.mesh, PartitionSpec("core"))
        in_specs = (PartitionSpec("core"),) * (n_params + n_outs)
        out_specs = (PartitionSpec("core"),) * n_outs
        self.fn = jax.jit(
            shard_map(_body, mesh=self.mesh, in_specs=in_specs,
                      out_specs=out_specs, check_rep=False),
            donate_argnums=tuple(range(n_params, n_params + n_outs)),
            keep_unused=True,
        )
        import jax.numpy as jnp
        self._mk_zeros = jax.jit(
            lambda: tuple(
                jnp.zeros((n_cores * a.shape[0], *a.shape[1:]), a.dtype)
                for a in out_avals),
            out_shardings=tuple(self.sharding for _ in out_avals),
        )

    def concat_inputs(self, in_maps):
        per_core = [[np.asarray(m[name]) for name in self.in_names]
                    for m in in_maps]
        return [
            np.concatenate([per_core[c][i] for c in range(self.n_cores)],
                           axis=0)
            for i in range(len(self.in_names))
        ]

    def put_inputs(self, in_maps):
        dev = [self.jax.device_put(a, self.sharding)
               for a in self.concat_inputs(in_maps)]
        self.jax.block_until_ready(dev)
        return dev

    def zeros(self):
        z = self._mk_zeros()
        self.jax.block_until_ready(z)
        return z

    def execute(self, dev_inputs):
        outs = self.fn(*dev_inputs, *self.zeros())
        self.jax.block_until_ready(outs)
        return outs

    def fetch(self, outs):
        """-> list of 8 per-core {name: np.ndarray}."""
        host = [np.asarray(o) for o in outs]
        return [
            {name: host[i].reshape(self.n_cores, *self.out_avals[i].shape)[c]
             for i, name in enumerate(self.out_names)}
            for c in range(self.n_cores)
        ]


def get_runner(key=(2048, 2048), rep=1, ver=2, **build_kw):
    ck = (key, rep, ver, tuple(sorted(build_kw.items())))
    if ck not in _RUNNER_CACHE:
        T, C = key
        builder = build_bass2 if ver == 2 else build_bass
        nc = builder(T=T, C=C, rep=rep, **build_kw)
        if rep == 1 and not build_kw:
            _NC_CACHE[key] = nc
        _RUNNER_CACHE[ck] = _Runner(nc)
    return _RUNNER_CACHE[ck]


def kernel(x, cos, sin, wq, wk, wv, wproj):
    x = np.asarray(x, dtype=np.float32)
    B, T, C = x.shape
    r = get_runner((T, C))
    in_maps = make_core_inputs(x, cos, sin, wq, wk, wv, wproj)
    dev = r.put_inputs(in_maps)
    res = r.fetch(r.execute(dev))
    out = np.zeros((B, T, C), dtype=np.float32)
    for b in range(B):
        for g in range(N_KV):
            out[b] += res[4 * b + g]["out"].astype(np.float32)
    return out

